# revision 23
# baseline (speedup 1.0000x reference)
"""Trainium2 Bass kernel for linear attention (ELU+1 feature map) block:
Q/K/V projections + linear attention + out-projection + residual + LayerNorm,
distributed over 8 NeuronCores.

Sharding: 8-way row split of the (batch*seq) dimension. Cores 2b and 2b+1
process the two 2048-row halves of batch b; the per-(batch,head) global
reductions KtQ^T [H,64,64] and q_sum [D] are pair-AllReduced on device.

v3 dataflow (PE-sequencer aware: every matmul streams >=512 moving columns
or uses self-loading f32r stationaries to keep dispatch off the critical path):
- Host pre-transposes X^T for all three projections (no PE transposes).
- Q/K projections run as fp8e4m3 DoubleRow matmuls (2 k-tiles, 0.5 cyc/col);
  weights arrive host-interleaved (pair, dout), scaled x16 out of the e4m3
  denormal range (undone by the ELU activation scale).
- KtQ^T accumulates head-PAIR blocks [128,128] (diagonal quadrants used)
  in one persistent PSUM bank pair across all 16 token subtiles; q_sum
  accumulates in a third bank. Single drain to the pair AllReduce.
- K stays SBUF-resident in fp8 for the phase-B normalizer.
- Phase B folds the output projection into the attention kernel:
  attn = (V . Z^-1_bcast) @ M with M_h = KtQ^T_h @ Wo_h built once per pair
  after the AllReduce; V path in f32r; output written in bf16.
"""
import os
import sys

for _p in ("/opt/trn_rl_repo", "/root/.axon_site/_ro/trn_rl_repo"):
    if os.path.isdir(_p) and _p not in sys.path:
        sys.path.insert(0, _p)

import numpy as np
import ml_dtypes

B, N, D, H = 4, 4096, 1024, 16
DEPTH = D // H  # 64
NCORES = 8
R = (B * N) // NCORES  # 2048 rows per core
NSUB = R // 128  # 16 token subtiles per core
NPAIR = NSUB // 2  # 8 subtile pairs
NGRP = NSUB // 4  # 4 groups of 4 subtiles (x^T load granularity)
NBLK = R // 512  # 4 token blocks per core
EPS_Z = 1e-9
EPS_LN = 1e-6
WSCALE = 16.0  # fp8 weight pre-scale (undone in the activation scale)

_CACHE = {}


def _build(zb_qk, zb_v, zb_o, g_one, b_zero, single=False):
    import concourse.bacc as bacc
    import concourse.tile as tile
    from concourse import mybir
    from concourse.masks import make_identity
    from contextlib import ExitStack
    from contextlib import ExitStack as _ES

    F32 = mybir.dt.float32
    F32R = mybir.dt.float32r
    BF16 = mybir.dt.bfloat16
    FP8 = mybir.dt.float8e4
    ALU = mybir.AluOpType
    AF = mybir.ActivationFunctionType
    DR = mybir.MatmulPerfMode.DoubleRow

    nc = bacc.Bacc("TRN2", debug=False, num_devices=1 if single else NCORES)

    xqT8 = nc.dram_tensor("xqT8", [D, R], FP8, kind="ExternalInput").ap()
    xkT8 = nc.dram_tensor("xkT8", [D, R], FP8, kind="ExternalInput").ap()
    xvT = nc.dram_tensor("xvT", [D, R], F32R, kind="ExternalInput").ap()
    xrb = nc.dram_tensor("xrb", [R, D], BF16, kind="ExternalInput").ap()
    wq8 = nc.dram_tensor("wq8", [512, 2048], FP8, kind="ExternalInput").ap()
    wk8 = nc.dram_tensor("wk8", [512, 2048], FP8, kind="ExternalInput").ap()
    wvr = nc.dram_tensor("wvr", [D, D], F32R, kind="ExternalInput").ap()
    wor = nc.dram_tensor("wor", [D, D], F32R, kind="ExternalInput").ap()
    bq = nc.dram_tensor("bq", [1, D], F32, kind="ExternalInput").ap()
    bk = nc.dram_tensor("bk", [1, D], F32, kind="ExternalInput").ap()
    bv = nc.dram_tensor("bv", [1, D], F32, kind="ExternalInput").ap()
    bo = nc.dram_tensor("bo", [1, D], F32, kind="ExternalInput").ap()
    bdm = nc.dram_tensor("bdm", [128, 1024], F32, kind="ExternalInput").ap()
    gamma = nc.dram_tensor("gamma", [1, D], F32, kind="ExternalInput").ap()
    beta = nc.dram_tensor("beta", [1, D], F32, kind="ExternalInput").ap()
    out = nc.dram_tensor("out", [R, D], BF16, kind="ExternalOutput").ap()

    with tile.TileContext(nc) as tc, ExitStack() as ctx:
        const_p = ctx.enter_context(tc.tile_pool(name="const", bufs=1))
        stage = ctx.enter_context(tc.tile_pool(name="stage", bufs=1))
        dp = ctx.enter_context(tc.tile_pool(name="dram", bufs=1, space="DRAM"))

        # ---- constants ----
        identb = const_p.tile([128, 128], BF16, tag="identb")
        idf = stage.tile([128, 128], F32, tag="idf", name="idf")
        make_identity(nc, idf[:])
        nc.scalar.copy(identb[:], idf[:])

        ones_f = const_p.tile([128, 1], F32, tag="ones_f")
        nc.gpsimd.memset(ones_f[:], 1.0)
        ones8 = const_p.tile([128, 64], FP8, tag="ones8")
        nc.gpsimd.memset(ones8[:], 1.0)

        eps_ln = const_p.tile([128, 1], F32, tag="eps_ln")
        nc.gpsimd.memset(eps_ln[:], EPS_LN)

        # S selection matrix: S[h, x] = 1 iff h == x // 64  (f32r)
        s_f = stage.tile([16, D], F32, tag="sstage", padded_shape=[128, D])
        nc.gpsimd.memset(s_f[:], 0.0)
        s_f3 = s_f[:].rearrange("h (j l) -> h j l", l=64)
        nc.gpsimd.affine_select(
            out=s_f3,
            in_=s_f3,
            compare_op=ALU.not_equal,
            fill=1.0,
            base=0,
            pattern=[[-1, 16], [0, 64]],
            channel_multiplier=1,
        )
        s_r = const_p.tile([16, D], F32R, tag="s_r")
        nc.scalar.copy(s_r[:], s_f[:])

        # 64x64 block-diagonal mask (zeroes KtQ head-pair cross quadrants)
        bdmask = const_p.tile([128, 1024], F32R, tag="bdmask")
        nc.gpsimd.dma_start(bdmask[:], bdm)

        def bcast_row(name, src_ap):
            row = const_p.tile([1, D], F32, tag=name + "_row")
            nc.sync.dma_start(row[:], src_ap)
            bc = const_p.tile([128, D], F32, tag=name + "_bc")
            nc.gpsimd.partition_broadcast(bc[:], row[:])
            return bc

        bq_bc = None if zb_qk else bcast_row("bq", bq)
        bk_bc = None if zb_qk else bcast_row("bk", bk)
        bo_bc = None if zb_o else bcast_row("bo", bo)
        gamma_bc = None if g_one else bcast_row("gamma", gamma)
        beta_bc = None if b_zero else bcast_row("beta", beta)
        bv_pp = None
        if not zb_v:
            # per-partition bias for feature-major V: bv_pp[p, c] = bv[c*128+p]
            bv_pp = const_p.tile([128, 8], F32, tag="bv_pp")
            for c in range(8):
                nc.sync.dma_start(
                    bv_pp[:, c : c + 1], bv[0:1, c * 128 : (c + 1) * 128]
                )

        cc_in = dp.tile([129, 1024], F32R, tag="cc_in")
        cc_out = dp.tile([129, 1024], F32R, tag="cc_out")

        # K cache: 8 pair tiles [tok 128, (sub 2) x (feat 1024)] fp8, resident
        kcache = ctx.enter_context(tc.tile_pool(name="kcache", bufs=1))
        kpairs = [
            kcache.tile([128, 2048], FP8, tag=f"kpair{p}", name=f"kpair{p}")
            for p in range(NPAIR)
        ]
        wB = ctx.enter_context(tc.tile_pool(name="wB", bufs=1))

        # =========================== PHASE A ===========================
        with _ES() as actx:
            wA = actx.enter_context(tc.tile_pool(name="wA", bufs=1))
            xT_p = actx.enter_context(tc.tile_pool(name="xTg", bufs=2))
            qp_p = actx.enter_context(tc.tile_pool(name="qpair", bufs=2))
            elu_t = actx.enter_context(tc.tile_pool(name="elu_t", bufs=4))
            drn = actx.enter_context(tc.tile_pool(name="drn", bufs=1))
            psA = actx.enter_context(tc.tile_pool(name="psA", bufs=4, space="PSUM"))
            psRed = actx.enter_context(tc.tile_pool(name="psRed", bufs=1, space="PSUM"))

            # prefetch x^T group 0 ahead of the weight tiles
            xqT4_0 = xT_p.tile([128, 4096], FP8, tag="xqT4", name="xqT4_0")
            nc.sync.dma_start(
                xqT4_0[:].rearrange("p (c tk) -> p c tk", c=8),
                xqT8.rearrange("(c p) tk -> p c tk", p=128)[:, :, 0:512],
            )
            xkT4_0 = xT_p.tile([128, 4096], FP8, tag="xkT4", name="xkT4_0")
            nc.scalar.dma_start(
                xkT4_0[:].rearrange("p (c tk) -> p c tk", c=8),
                xkT8.rearrange("(c p) tk -> p c tk", p=128)[:, :, 0:512],
            )
            # fp8 DR pair weights: 4 tiles [128, (t 2) x (dout 1024)]
            wq_t = []
            wk_t = []
            for c in range(4):
                wt = wA.tile([128, 2048], FP8, tag=f"wq{c}", name=f"wq{c}")
                nc.sync.dma_start(wt[:], wq8[c * 128 : (c + 1) * 128, :])
                wq_t.append(wt)
                wt = wA.tile([128, 2048], FP8, tag=f"wk{c}", name=f"wk{c}")
                nc.scalar.dma_start(wt[:], wk8[c * 128 : (c + 1) * 128, :])
                wk_t.append(wt)
            # f32r V weights (phase B; gpsimd DMA queue, off the critical path)
            wv_t = []
            for kk in range(8):
                wt = wB.tile([128, D], F32R, tag=f"wv{kk}", name=f"wv{kk}")
                nc.gpsimd.dma_start(wt[:], wvr[kk * 128 : (kk + 1) * 128, :])
                wv_t.append(wt)

            # persistent psum accumulators
            ktq_ps = psRed.tile([128, 1024], F32, tag="ktq_ps")
            qs_ps0 = psRed.tile([128, 512], F32, tag="qs_ps0")
            qs_ps1 = psRed.tile([128, 512], F32, tag="qs_ps1")
            qs_ps = [qs_ps0, qs_ps1]

            def project_dr(vT, u, w_tiles, dst, dst_off, bias_bc, name,
                           a1_on_act=True):
                """dst[:, dst_off:dst_off+1024] (fp8) = elu(x @ W + b) + 1.

                vT: x^T group view [128, c 8, tok 512]; u: subtile within group.
                """
                for of in range(2):
                    ph = psA.tile([128, 512], F32, tag="ps_p", name=f"pp_{name}{of}")
                    for c in range(4):
                        lhs = vT[:, 2 * c : 2 * c + 2, u * 128 : (u + 1) * 128]
                        rhs = w_tiles[c][:].rearrange("p (t m) -> p t m", t=2)[
                            :, :, of * 512 : (of + 1) * 512
                        ]
                        nc.tensor.matmul(
                            ph[:], lhs, rhs, start=(c == 0), stop=(c == 3),
                            perf_mode=DR,
                        )
                    dsl = dst[:, dst_off + of * 512 : dst_off + (of + 1) * 512]
                    if bias_bc is None:
                        e = elu_t.tile([128, 512], BF16, tag="e")
                        nc.scalar.activation(e[:], ph[:], AF.Exp, scale=1.0 / WSCALE)
                        a1 = elu_t.tile([128, 512], BF16, tag="a1")
                        if a1_on_act:
                            nc.scalar.activation(
                                a1[:], ph[:], AF.Identity,
                                bias=ones_f[:], scale=1.0 / WSCALE,
                            )
                        else:
                            nc.vector.tensor_scalar(
                                a1[:], ph[:], 1.0 / WSCALE, 1.0,
                                op0=ALU.mult, op1=ALU.add,
                            )
                        nc.vector.scalar_tensor_tensor(
                            dsl, e[:], 1.0, a1[:], op0=ALU.min, op1=ALU.max
                        )
                    else:
                        sl = slice(of * 512, (of + 1) * 512)
                        pre = elu_t.tile([128, 512], F32, tag="pre")
                        nc.vector.scalar_tensor_tensor(
                            pre[:], ph[:], 1.0 / WSCALE, bias_bc[:, sl],
                            op0=ALU.mult, op1=ALU.add,
                        )
                        e = elu_t.tile([128, 512], BF16, tag="e")
                        nc.scalar.activation(e[:], pre[:], AF.Exp)
                        a1 = elu_t.tile([128, 512], BF16, tag="a1")
                        nc.scalar.activation(
                            a1[:], pre[:], AF.Identity, bias=ones_f[:]
                        )
                        nc.vector.scalar_tensor_tensor(
                            dsl, e[:], 1.0, a1[:], op0=ALU.min, op1=ALU.max
                        )

            def reduce_pair(pair, q_pair, k_pair):
                """KtQ^T head-pair diagonal blocks += Q^T K; q_sum += 1^T Q."""
                q3 = q_pair[:].rearrange("p (t f) -> p t f", t=2)
                k3 = k_pair[:].rearrange("p (t f) -> p t f", t=2)
                first = pair == 0
                last = pair == NPAIR - 1
                for hp in range(8):
                    nc.tensor.matmul(
                        ktq_ps[:, hp * 128 : (hp + 1) * 128],
                        q3[:, :, hp * 128 : (hp + 1) * 128],
                        k3[:, :, hp * 128 : (hp + 1) * 128],
                        start=(first and hp in (0, 4)),
                        stop=last,
                        perf_mode=DR,
                        skip_group_check=True,
                    )
                for of in range(2):
                    nc.tensor.matmul(
                        qs_ps[of][0:32, :],
                        ones8[:].rearrange("p (t m) -> p t m", t=2),
                        q3[:, :, of * 512 : (of + 1) * 512],
                        start=first,
                        stop=last,
                        perf_mode=DR,
                        skip_group_check=True,
                        tile_position=(0, 0),
                    )

            pending = None  # KtQ/q_sum reductions trail by one pair
            for g in range(NGRP):
                cols = slice(g * 512, (g + 1) * 512)
                if g == 0:
                    xqT4, xkT4 = xqT4_0, xkT4_0
                else:
                    xqT4 = xT_p.tile([128, 4096], FP8, tag="xqT4",
                                     name=f"xqT4_{g}")
                    nc.sync.dma_start(
                        xqT4[:].rearrange("p (c tk) -> p c tk", c=8),
                        xqT8.rearrange("(c p) tk -> p c tk", p=128)[:, :, cols],
                    )
                    xkT4 = xT_p.tile([128, 4096], FP8, tag="xkT4",
                                     name=f"xkT4_{g}")
                    nc.scalar.dma_start(
                        xkT4[:].rearrange("p (c tk) -> p c tk", c=8),
                        xkT8.rearrange("(c p) tk -> p c tk", p=128)[:, :, cols],
                    )
                vq = xqT4[:].rearrange("p (c tk) -> p c tk", c=8)
                vk = xkT4[:].rearrange("p (c tk) -> p c tk", c=8)
                for u in range(4):
                    s = 4 * g + u
                    pair, t = s // 2, s % 2
                    if t == 0:
                        q_pair = qp_p.tile([128, 2048], FP8, tag="q_pair",
                                           name=f"qp{pair}")
                    k_pair = kpairs[pair]
                    project_dr(vq, u, wq_t, q_pair, t * 1024, bq_bc,
                               f"q{s}", a1_on_act=True)
                    project_dr(vk, u, wk_t, k_pair, t * 1024, bk_bc,
                               f"k{s}", a1_on_act=False)
                    if t == 1:
                        if pending is not None:
                            reduce_pair(*pending)
                        pending = (pair, q_pair, k_pair)

            reduce_pair(*pending)

            # drain reductions to DRAM for the pair AllReduce
            ktq_stage = drn.tile([128, 1024], F32R, tag="ktq_stage")
            nc.scalar.copy(ktq_stage[:], ktq_ps[:])
            nc.sync.dma_start(cc_in[0:128, :], ktq_stage[:])
            qs_stage = drn.tile([1, 1024], F32R, tag="qs_stage",
                                padded_shape=[128, 1024])
            nc.scalar.copy(qs_stage[0:1, 0:512], qs_ps0[0:1, :])
            nc.scalar.copy(qs_stage[0:1, 512:1024], qs_ps1[0:1, :])
            nc.sync.dma_start(cc_in[128:129, :], qs_stage[0:1, :])

        if single:
            nc.sync.dma_start(cc_out[:], cc_in[:])
        else:
            nc.gpsimd.collective_compute(
                "AllReduce",
                mybir.AluOpType.add,
                replica_groups=[[0, 1], [2, 3], [4, 5], [6, 7]],
                ins=[cc_in.opt()],
                outs=[cc_out.opt()],
            )

        # =========================== PHASE B ===========================
        with _ES() as bctx:
            wBo = bctx.enter_context(tc.tile_pool(name="wBo", bufs=1))
            redu = bctx.enter_context(tc.tile_pool(name="redu", bufs=1))
            msb_p = bctx.enter_context(tc.tile_pool(name="msb", bufs=1))
            xbnat = bctx.enter_context(tc.tile_pool(name="xbnat", bufs=3))
            xvT_p = bctx.enter_context(tc.tile_pool(name="xvT", bufs=8))
            vsb_p = bctx.enter_context(tc.tile_pool(name="vsb", bufs=8))
            zt_p = bctx.enter_context(tc.tile_pool(name="zt", bufs=2))
            inv_p = bctx.enter_context(tc.tile_pool(name="invz", bufs=4))
            lnt = bctx.enter_context(tc.tile_pool(name="lnt", bufs=2))
            small = bctx.enter_context(tc.tile_pool(name="small", bufs=4))
            ostage = bctx.enter_context(tc.tile_pool(name="ostage", bufs=2))
            psV = bctx.enter_context(tc.tile_pool(name="psV", bufs=3, space="PSUM"))
            psBig = bctx.enter_context(tc.tile_pool(name="psBig", bufs=3, space="PSUM"))
            psTrB = bctx.enter_context(tc.tile_pool(name="psTrB", bufs=1, space="PSUM"))
            psDiv = bctx.enter_context(tc.tile_pool(name="psDiv", bufs=1, space="PSUM"))

            wo_t = []
            for c in range(8):
                wt = wBo.tile([128, D], F32R, tag=f"wo{c}", name=f"wo{c}")
                nc.gpsimd.dma_start(wt[:], wor[c * 128 : (c + 1) * 128, :])
                wo_t.append(wt)

            def phase_b_setup():
                # KtQ^T (f32r) and q_sum broadcast from the AllReduce result
                ktqb = redu.tile([128, 1024], F32R, tag="ktqb")
                nc.gpsimd.dma_start(ktqb[:], cc_out[0:128, :])
                nc.vector.tensor_tensor(ktqb[:], ktqb[:], bdmask[:], ALU.mult)

                qsum_lin = stage.tile([1, 1024], F32R, tag="qsum_stage",
                                      name="qsum_lin", padded_shape=[128, 1024])
                nc.gpsimd.dma_start(qsum_lin[:], cc_out[128:129, :])
                qsum_bc = redu.tile([128, 1024], F32R, tag="qsum_bc")
                nc.gpsimd.partition_broadcast(qsum_bc[:], qsum_lin[:])

                # M[d, n] = sum_{e in head(d)} KtQ[d, e] Wo[e, n]  (f32r)
                # head-pair hp: diag quadrants of ktqb[:, hp*128:(hp+1)*128]
                m_sb = []
                for hp in range(8):
                    mt = msb_p.tile([128, 1024], F32R, tag=f"m{hp}",
                                    name=f"m{hp}")
                    for of in range(2):
                        psM = psBig.tile([128, 512], F32, tag="ps_big",
                                         name=f"psm_{hp}_{of}")
                        nc.tensor.matmul(
                            psM[:],
                            ktqb[:, hp * 128 : (hp + 1) * 128],
                            wo_t[hp][:, of * 512 : (of + 1) * 512],
                            start=True, stop=True,
                        )
                        nc.scalar.copy(
                            mt[:, of * 512 : (of + 1) * 512], psM[:]
                        )
                    m_sb.append(mt)
                return m_sb, qsum_bc

            def compute_invz_blk(qsum_bc, blk):
                """Z = K . q_sum for one 512-token block -> invz_fm tile."""
                if True:
                    fm = inv_p.tile([16, 512], F32R, tag="invz_fm",
                                    padded_shape=[128, 512],
                                    name=f"invz{blk}")
                    for t in range(4):
                        s = blk * 4 + t
                        ksl = kpairs[s // 2][:, (s % 2) * 1024 : (s % 2 + 1) * 1024]
                        prod = zt_p.tile([128, 1024], BF16, tag="prod")
                        if t % 2 == 0:
                            nc.gpsimd.tensor_tensor(prod[:], ksl, qsum_bc[:],
                                                    ALU.mult)
                        else:
                            nc.vector.tensor_tensor(prod[:], ksl, qsum_bc[:],
                                                    ALU.mult)
                        z_t = zt_p.tile([128, 16], BF16, tag="z_t")
                        with nc.allow_low_precision(reason="Z is O(4e3), bf16 ok"):
                            nc.vector.tensor_reduce(
                                z_t[:],
                                prod[:].rearrange("p (h e) -> p h e", e=64),
                                mybir.AxisListType.X,
                                ALU.add,
                            )
                            iz_t = zt_p.tile([128, 16], BF16, tag="iz_t")
                            nc.vector.reciprocal(iz_t[:], z_t[:])
                        ps_zt = psTrB.tile([128, 512], BF16, tag="trB",
                                           name=f"trz_{blk}_{t}")
                        nc.tensor.transpose(ps_zt[0:16, 0:128], iz_t[:],
                                            identb[:])
                        nc.scalar.copy(
                            fm[:, t * 128 : (t + 1) * 128], ps_zt[0:16, 0:128]
                        )
                    return fm

            m_sb = qsum_bc = None
            invz = [None] * NBLK
            for blk in range(NBLK):
                # ---- x_v^T load (f32r, pre-transposed on host) ----
                xv_t = []
                for kk in range(8):
                    xt = xvT_p.tile([128, 512], F32R, tag="xvT",
                                    name=f"xvt_{blk}_{kk}")
                    nc.sync.dma_start(
                        xt[:],
                        xvT[kk * 128 : (kk + 1) * 128,
                            blk * 512 : (blk + 1) * 512],
                    )
                    xv_t.append(xt)

                if blk == 0:
                    m_sb, qsum_bc = phase_b_setup()
                    invz[0] = compute_invz_blk(qsum_bc, 0)
                invz_fm = invz[blk]

                # ---- V' = (xv @ Wv [+ bv]) * invz_bcast  (feature-major) ----
                v_sb = []
                for c in range(8):
                    ps_d = psDiv.tile([128, 512], F32, tag="ps_d")
                    nc.tensor.matmul(
                        ps_d[:],
                        s_r[:, c * 128 : (c + 1) * 128],
                        invz_fm[:],
                        start=True, stop=True,
                    )
                    div_sb = zt_p.tile([128, 512], F32, tag="div_sb")
                    nc.scalar.copy(div_sb[:], ps_d[:])
                    ps_v = psV.tile([128, 512], F32, tag="ps_v")
                    for kk in range(8):
                        nc.tensor.matmul(
                            ps_v[:],
                            wv_t[kk][:, c * 128 : (c + 1) * 128],
                            xv_t[kk][:],
                            start=(kk == 0),
                            stop=(kk == 7),
                        )
                    vt = vsb_p.tile([128, 512], F32R, tag="v_sb",
                                    name=f"v_{blk}_{c}")
                    if not zb_v:
                        vb_t = zt_p.tile([128, 512], F32, tag="vb")
                        nc.scalar.activation(
                            vb_t[:], ps_v[:], AF.Identity, bias=bv_pp[:, c : c + 1]
                        )
                        src_v = vb_t[:]
                    else:
                        src_v = ps_v[:]
                    if blk == 0:
                        # first block: drain V before the AllReduce-dependent
                        # divisor exists, so the PE overlaps the phase switch
                        nc.scalar.copy(vt[:], src_v)
                        nc.vector.tensor_tensor(vt[:], vt[:], div_sb[:], ALU.mult)
                    else:
                        nc.vector.scalar_tensor_tensor(
                            vt[:], src_v, 1.0, div_sb[:],
                            op0=ALU.mult, op1=ALU.mult,
                        )
                    v_sb.append(vt)

                if blk + 1 < NBLK:
                    invz[blk + 1] = compute_invz_blk(qsum_bc, blk + 1)

                # ---- attn = V' @ M, + residual, LayerNorm ----
                for t in range(4):
                    rows = slice(blk * 512 + t * 128, blk * 512 + (t + 1) * 128)
                    r_nat = xbnat.tile([128, 1024], BF16, tag="xr_nat",
                                       name=f"xr_{blk}_{t}")
                    nc.sync.dma_start(r_nat[:], xrb[rows, :])
                    res = r_nat[:]
                    if bo_bc is not None:
                        qb_t = lnt.tile([128, 1024], F32, tag="qb")
                        nc.vector.tensor_tensor(qb_t[:], r_nat[:], bo_bc[:],
                                                ALU.add)
                        res = qb_t[:]

                    x_sb = lnt.tile([128, 1024], BF16, tag="x_sb")
                    s1 = small.tile([128, 2], F32, tag="s1")
                    for of in range(2):
                        sl = slice(of * 512, (of + 1) * 512)
                        ps_a = psBig.tile([128, 512], F32, tag="ps_big",
                                          name=f"ps_a_{blk}_{t}_{of}")
                        for c in range(8):
                            nc.tensor.matmul(
                                ps_a[:],
                                v_sb[c][:, t * 128 : (t + 1) * 128],
                                m_sb[c][:, of * 512 : (of + 1) * 512],
                                start=(c == 0),
                                stop=(c == 7),
                            )
                        nc.vector.scalar_tensor_tensor(
                            x_sb[:, sl], ps_a[:], 1.0, res[:, sl],
                            op0=ALU.mult, op1=ALU.add,
                            accum_out=s1[:, of : of + 1],
                        )
                    s1t = small.tile([128, 1], F32, tag="s1t")
                    nc.vector.tensor_reduce(
                        s1t[:], s1[:], mybir.AxisListType.X, ALU.add
                    )
                    mu = small.tile([128, 1], F32, tag="mu")
                    nc.scalar.mul(mu[:], s1t[:], 1.0 / D)
                    y = ostage.tile([128, 1024], BF16, tag="y")
                    s2 = small.tile([128, 1], F32, tag="s2")
                    # y is scratch here; overwritten below
                    nc.scalar.activation(y[:], x_sb[:], AF.Square, accum_out=s2[:])
                    mu2 = small.tile([128, 1], F32, tag="mu2")
                    nc.scalar.square(mu2[:], mu[:])
                    var = small.tile([128, 1], F32, tag="var")
                    nc.vector.tensor_scalar(
                        var[:], s2[:], 1.0 / D, mu2[:], op0=ALU.mult,
                        op1=ALU.subtract,
                    )
                    std = small.tile([128, 1], F32, tag="std")
                    nc.scalar.activation(std[:], var[:], AF.Sqrt, bias=eps_ln[:])
                    rstd = small.tile([128, 1], F32, tag="rstd")
                    nc.vector.reciprocal(rstd[:], std[:])

                    nc.vector.tensor_scalar(
                        y[:], x_sb[:], mu[:], rstd[:],
                        op0=ALU.subtract, op1=ALU.mult,
                    )
                    if not g_one:
                        nc.vector.tensor_tensor(y[:], y[:], gamma_bc[:], ALU.mult)
                    if not b_zero:
                        nc.vector.tensor_tensor(y[:], y[:], beta_bc[:], ALU.add)
                    nc.sync.dma_start(out[rows, :], y[:])

    nc.compile()
    return nc


def _get_nc(flags):
    if flags not in _CACHE:
        _CACHE[flags] = _build(*flags)
    return _CACHE[flags]


def _prep(inputs):
    E4M3 = ml_dtypes.float8_e4m3
    BF16 = ml_dtypes.bfloat16
    q = np.ascontiguousarray(np.asarray(inputs["query"], dtype=np.float32))
    k = np.ascontiguousarray(np.asarray(inputs["key"], dtype=np.float32))
    v = np.ascontiguousarray(np.asarray(inputs["value"], dtype=np.float32))
    Wq = np.asarray(inputs["Wq"], dtype=np.float32)
    Wk = np.asarray(inputs["Wk"], dtype=np.float32)
    Wv = np.asarray(inputs["Wv"], dtype=np.float32)
    Wo = np.asarray(inputs["Wo"], dtype=np.float32)
    bqv = np.ascontiguousarray(np.asarray(inputs["bq"], dtype=np.float32).reshape(1, D))
    bkv = np.ascontiguousarray(np.asarray(inputs["bk"], dtype=np.float32).reshape(1, D))
    bvv = np.ascontiguousarray(np.asarray(inputs["bv"], dtype=np.float32).reshape(1, D))
    bov = np.ascontiguousarray(np.asarray(inputs["bo"], dtype=np.float32).reshape(1, D))
    gv = np.ascontiguousarray(np.asarray(inputs["gamma"], dtype=np.float32).reshape(1, D))
    btv = np.ascontiguousarray(np.asarray(inputs["beta"], dtype=np.float32).reshape(1, D))

    flags = (
        bool(not bqv.any() and not bkv.any()),
        bool(not bvv.any()),
        bool(not bov.any()),
        bool(np.all(gv == 1.0)),
        bool(not btv.any()),
    )

    def pack_pairs(W):
        # [512, 2048]: row c*128+p, col t*1024+m = W[(2c+t)*128+p, m] * WSCALE
        Ws = (W * WSCALE).reshape(8, 128, D)
        pairs = np.concatenate(
            [np.concatenate([Ws[2 * c], Ws[2 * c + 1]], axis=1) for c in range(4)],
            axis=0,
        )
        return np.ascontiguousarray(pairs).astype(E4M3)

    wq8 = pack_pairs(Wq)
    wk8 = pack_pairs(Wk)
    wvr = np.ascontiguousarray(Wv)
    wor = np.ascontiguousarray(Wo)

    bdm_np = np.zeros((128, 128), np.float32)
    bdm_np[:64, :64] = 1.0
    bdm_np[64:, 64:] = 1.0
    bdm_np = np.ascontiguousarray(np.tile(bdm_np, (1, 8)))

    qf = q.reshape(NCORES, R, D)
    kf = k.reshape(NCORES, R, D)
    vf = v.reshape(NCORES, R, D)
    in_maps = []
    for c in range(NCORES):
        in_maps.append(
            {
                "xqT8": np.ascontiguousarray(qf[c].T).astype(E4M3),
                "xkT8": np.ascontiguousarray(kf[c].T).astype(E4M3),
                "xvT": np.ascontiguousarray(vf[c].T),
                "xrb": qf[c].astype(BF16),
                "wq8": wq8, "wk8": wk8, "wvr": wvr, "wor": wor,
                "bdm": bdm_np,
                "bq": bqv, "bk": bkv, "bv": bvv, "bo": bov,
                "gamma": gv, "beta": btv,
            }
        )
    return flags, in_maps


def kernel(**inputs):
    from concourse.bass_utils import run_bass_kernel_spmd

    flags, in_maps = _prep(inputs)
    nc = _get_nc(flags)
    res = run_bass_kernel_spmd(nc, in_maps, core_ids=list(range(NCORES)))
    outs = np.stack(
        [np.asarray(res.results[c]["out"]).astype(np.float32) for c in range(NCORES)],
        axis=0,
    )
    return outs.reshape(B, N, D)
